# revision 3
# baseline (speedup 1.0000x reference)
"""Trainium2 Bass kernel v3 for nn_DirectDepthMapper (histogram binning).

Like the baseline (contiguous one-hot builds, per-pixel-column matmuls) but:
  * operands swapped: stationary = X one-hot [128, wgn<=21] (tiny ldweights),
    moving = W one-hot [128, 40];
  * PSUM column-group tiling: pixel-column f -> tile_position (0, 32*(f%4)),
    so 4 matmuls execute concurrently in the 4 col-group quadrants;
  * quantize chain fused on ACT (round via +-2^23 bias), Y = ix-lo rebase,
    one-hot builds split across DVE (4x bf16 tensor_scalar) and Pool.
Host sums the 4 col-group slots + 8 cores, scatters group windows.
"""

import sys

sys.path.insert(0, "/opt/trn_rl_repo")

import numpy as np

H = 4096
W = 4096
GRID_N = 400
NEAR_TH = np.float32(0.1)

IZ_LO, IZ_N = 201, 40
IX_LO, IX_N = 180, 41

ROWS_USED = 2048
N_CORES = 8
RPC = ROWS_USED // N_CORES
PB = 128
NBLK = RPC // PB
FG = 128
NG = W // FG
FW = 512
NWG = W // FW
REPEAT = 1

W_DVE_ROWS = 40
SKIP_PE = False
MM_STRIDE = 1
U_ON_DVE = True


def _stack_of(wmx):
    return 8 if wmx <= 16 else 4

TWO23 = np.float32(8388608.0)
B23 = float(200.0 + 8388608.0)
XSCALE = np.float32(10.0 / 4096.0)


_PATCHED = False


def _apply_tile_patch():
    global _PATCHED
    if _PATCHED:
        return
    import concourse.tile as tile_mod
    from concourse.vector_clock import ScopedClock, VectorClock

    n_procs = 27

    def _drain_and_barrier(self, tick_clock, wait_clock):
        g = tick_clock.global_clock
        procs = [p for p in range(n_procs) if g[p] > 0]
        for p in procs:
            vc = VectorClock([g[q] if q == p else 0 for q in range(n_procs)])
            d = self.nc.sync.drain()
            wait_clock.add_sem_waits(d.ins, ScopedClock({None: vc}))
        self.nc.all_engine_barrier()
        assert self.sems is not None
        popped = self.nc._tile_sem_poison_stack.pop()
        assert popped is self._sem_poison
        self.nc.clear_and_free_semaphores(list(self.sems.allocated().values()))
        self.nc.all_engine_barrier()

    tile_mod.TileContext._drain_and_barrier = _drain_and_barrier
    _PATCHED = True


def _split_multiwait(nc):
    import concourse.mybir as mybir

    for f in nc.m.functions:
        for bb in f.blocks:
            out = []
            changed = False
            for inst in bb.instructions:
                si = inst.sync_info
                waits = list(si.on_wait) if si is not None and si.on_wait else []
                if len(waits) > 1:
                    changed = True
                    for wi, w in enumerate(waits[:-1]):
                        nop = mybir.InstNoOp(
                            name=f"{inst.name}-sw{wi}",
                            engine=inst.engine,
                            sync_info=mybir.SyncInfo(on_wait=[w], on_update=[]),
                            bass_nofuse=True,
                        )
                        out.append(nop)
                    si.on_wait = [waits[-1]]
                out.append(inst)
            if changed:
                bb.instructions[:] = out


def _tj_table():
    tj = np.empty(ROWS_USED, np.float32)
    for j in range(ROWS_USED):
        m = 2047 - j
        if m <= 0:
            tj[j] = np.float32(-1.0)
            continue
        if m <= 1024:
            tj[j] = np.float32(4.0)
            continue
        t = np.float32(np.float64(4096.0) / m)
        while np.float32(t * np.float32(m)) >= np.float32(4096.0):
            t = np.nextafter(t, np.float32(-np.inf), dtype=np.float32)
        up = np.nextafter(t, np.float32(np.inf), dtype=np.float32)
        while np.float32(up * np.float32(m)) < np.float32(4096.0):
            t = up
            up = np.nextafter(t, np.float32(np.inf), dtype=np.float32)
        tj[j] = min(np.float32(4.0), up)
    return tj


def _x_windows():
    wins = []
    for g in range(NG):
        ci_min = g * FG - 2047
        ci_max = g * FG + FG - 1 - 2047
        glo = 200.0 + min(0.0, 40.0 * ci_min / 4096.0, 40.0 * ci_max / 4096.0)
        ghi = 200.0 + max(0.0, 40.0 * ci_min / 4096.0, 40.0 * ci_max / 4096.0)
        lo = int(np.clip(np.floor(glo), IX_LO, IX_LO + IX_N - 1))
        hi = int(np.clip(np.ceil(ghi), IX_LO, IX_LO + IX_N - 1))
        wins.append((lo, hi - lo + 1))
    return wins


def _build_bass():
    import concourse.bass as bass
    import concourse.mybir as mybir
    import concourse.tile as tile
    from contextlib import ExitStack

    dt = mybir.dt
    op = mybir.AluOpType
    AF = mybir.ActivationFunctionType

    nc = bass.Bass("TRN2", target_bir_lowering=False, debug=False)

    depth_in = nc.dram_tensor(
        "depth_slice", [RPC, W], dt.float32, kind="ExternalInput"
    )
    tj_in = nc.dram_tensor("tj", [RPC, 1], dt.float32, kind="ExternalInput")
    part_out = nc.dram_tensor(
        "partial", [NG, PB, IZ_N * 8], dt.float32, kind="ExternalOutput"
    )

    wins = _x_windows()
    wmax_wg = [
        max(wins[wg * (FW // FG) + k][1] for k in range(FW // FG))
        for wg in range(NWG)
    ]

    with tile.TileContext(nc) as tc:
        with ExitStack() as ctx:
            cpool = ctx.enter_context(tc.tile_pool(name="consts", bufs=1))
            dpool = ctx.enter_context(tc.tile_pool(name="depth", bufs=2))
            epool = ctx.enter_context(tc.tile_pool(name="etmp", bufs=2))
            bpool = ctx.enter_context(tc.tile_pool(name="bins", bufs=2))
            wpool = ctx.enter_context(tc.tile_pool(name="wonehot", bufs=2))
            # xt feeds only the same-engine (DVE) repack: bufs=1 is stall-free
            xpool = ctx.enter_context(tc.tile_pool(name="xonehot", bufs=1))
            ppool = ctx.enter_context(
                tc.tile_pool(name="psum", bufs=1, space="PSUM")
            )
            gpool = ctx.enter_context(tc.tile_pool(name="stage", bufs=2))
            spool2 = ctx.enter_context(tc.tile_pool(name="xstack", bufs=2))

            cx = cpool.tile([PB, W], dt.float32)
            lo_full = cpool.tile([PB, W], dt.bfloat16)
            with tc.tile_pool(name="setup", bufs=1) as spool:
                cx_i16 = spool.tile([PB, W], dt.int16)
                nc.gpsimd.iota(
                    cx_i16[:, :], pattern=[[1, W]], base=-2047,
                    channel_multiplier=0,
                )
                nc.vector.tensor_copy(cx[:, :], cx_i16[:, :])
            for g in range(NG):
                nc.gpsimd.memset(
                    lo_full[:, g * FG : (g + 1) * FG], float(wins[g][0])
                )

            tjs = []
            for b in range(NBLK):
                tjt = dpool.tile([PB, 1], dt.float32, tag=f"tj{b}")
                nc.sync.dma_start(
                    tjt[:, :], tj_in.ap()[b * PB : (b + 1) * PB, :]
                )
                tjs.append(tjt)

            psums = []
            for k in range(8):
                ps = ppool.tile(
                    [PB, IZ_N * 8], dt.float32, space="PSUM", tag=f"ps{k}",
                    name=f"psbank{k}",
                )
                if SKIP_PE:
                    nc.vector.memset(ps[:, :], 0.0)
                psums.append(ps)

            for rep in range(REPEAT):
                for wg in range(NWG):
                    wsl = slice(wg * FW, (wg + 1) * FW)
                    wmx = wmax_wg[wg]
                    S = _stack_of(wmx)
                    nstk = FW // S
                    spg = FG // S
                    for b in range(NBLK):
                        d = dpool.tile([PB, FW], dt.float32, tag=f"d{b}")
                        nc.sync.dma_start(
                            d[:, :],
                            depth_in.ap()[b * PB : (b + 1) * PB, wsl],
                        )
                        dsl = slice(0, FW)
                        u = epool.tile([PB, FW], dt.float32, tag="u")
                        ueng = nc.vector if U_ON_DVE else nc.gpsimd
                        ueng.tensor_tensor(
                            out=u[:, :], in0=d[:, dsl], in1=cx[:, wsl],
                            op=op.mult,
                        )
                        nc.scalar.activation(
                            u[:, :], u[:, :], AF.Copy, bias=B23,
                            scale=float(XSCALE),
                        )
                        sx = bpool.tile([PB, FW], dt.bfloat16, tag="sx")
                        nc.scalar.activation(
                            sx[:, :], u[:, :], AF.Copy, bias=-float(TWO23),
                            scale=1.0,
                        )
                        yt = bpool.tile([PB, FW], dt.bfloat16, tag="yt")
                        nc.vector.tensor_tensor(
                            out=yt[:, :], in0=sx[:, :], in1=lo_full[:, wsl],
                            op=op.subtract,
                        )
                        zq = epool.tile([PB, FW], dt.float32, tag="zq")
                        nc.scalar.activation(
                            zq[:, :], d[:, dsl], AF.Copy, bias=B23, scale=10.0
                        )
                        nc.scalar.activation(
                            zq[:, :], zq[:, :], AF.Copy, bias=-float(TWO23),
                            scale=1.0,
                        )
                        nc.vector.scalar_tensor_tensor(
                            out=zq[:, :], in0=d[:, dsl], scalar=tjs[b][:, 0:1],
                            in1=zq[:, :], op0=op.is_lt, op1=op.mult,
                        )
                        sz = bpool.tile([PB, FW], dt.bfloat16, tag="sz")
                        nc.vector.scalar_tensor_tensor(
                            out=sz[:, :], in0=d[:, dsl], scalar=float(NEAR_TH),
                            in1=zq[:, :], op0=op.is_ge, op1=op.mult,
                        )

                        wt = wpool.tile([PB, IZ_N, FW], dt.bfloat16, tag="wt")
                        for r in range(W_DVE_ROWS):
                            nc.vector.tensor_scalar(
                                wt[:, r, :], sz[:, :], float(IZ_LO + r), None,
                                op.is_equal,
                            )
                        for r in range(W_DVE_ROWS, IZ_N):
                            nc.gpsimd.tensor_scalar(
                                wt[:, r, :], sz[:, :], float(IZ_LO + r), None,
                                op.is_equal,
                            )
                        xt = xpool.tile([PB, 21, FW], dt.bfloat16, tag="xt")
                        for w in range(wmx):
                            nc.vector.tensor_scalar(
                                xt[:, w, :], yt[:, :], float(w), None,
                                op.is_equal,
                            )

                        if SKIP_PE:
                            continue
                        # repack X one-hot to stack-major: xs[p, st, w, f]
                        # = xt[p, w, st*S+f]; stationary slice xs[:, st]
                        # is then one uniform-stride free dim.
                        xs = spool2.tile(
                            [PB, nstk, wmx, S], dt.bfloat16, tag="xs"
                        )
                        nc.vector.tensor_copy(
                            xs[:, :, :, :],
                            xt[:, 0:wmx, :].rearrange(
                                "p w (st f) -> p st w f", f=S
                            ),
                        )
                        # stacked matmuls: lhsT = xs[:, st] (S pixel-cols,
                        # w-major), rhs = wt[:, :, f0:f0+S] (2 free dims),
                        # psum rows w*S+f, cols r*S+f2; diag f==f2 real.
                        for k in range(FW // FG):
                            g = wg * (FW // FG) + k
                            lo, wgn = wins[g]
                            ps = psums[g % 8]
                            sis = [s for s in range(spg)
                                   if s % MM_STRIDE == 0]
                            for si in sis:
                                st = k * spg + si
                                f0 = st * S
                                nc.tensor.matmul(
                                    out=ps[: S * wgn, : S * IZ_N],
                                    lhsT=xs[:, st, 0:wgn, :],
                                    rhs=wt[:, :, f0 : f0 + S],
                                    start=(
                                        rep == 0 and b == 0 and si == sis[0]
                                    ),
                                    stop=(
                                        rep == REPEAT - 1
                                        and b == NBLK - 1
                                        and si == sis[-1]
                                    ),
                                )
                            if rep == REPEAT - 1 and b == NBLK - 1:
                                stage = gpool.tile(
                                    [PB, IZ_N * 8], dt.float32, tag="st"
                                )
                                nc.scalar.activation(
                                    stage[: S * wgn, : S * IZ_N],
                                    ps[: S * wgn, : S * IZ_N],
                                    AF.Copy, bias=0.0, scale=1.0,
                                )
                                nc.sync.dma_start(
                                    part_out.ap()[g, : S * wgn, : S * IZ_N],
                                    stage[: S * wgn, : S * IZ_N],
                                )

    _split_multiwait(nc)
    return nc


_NC_CACHE = None


def _get_nc():
    global _NC_CACHE
    if _NC_CACHE is None:
        _apply_tile_patch()
        _NC_CACHE = _build_bass()
    return _NC_CACHE


def _numpy_reference(depth, pose):
    from math import ceil, floor

    h, w = depth.shape
    fx, fy = np.float32(w), np.float32(h)
    cx, cy = w // 2 - 1, h // 2 - 1

    d = depth.T.reshape(-1)
    xv = np.repeat(np.arange(w, dtype=np.float32), h)
    yv = np.tile(np.arange(h, dtype=np.float32), w)
    X = d * (xv - np.float32(cx)) / fx
    Y = d * (yv - np.float32(cy)) / fy
    Z = d

    mask = (np.abs(Z) < np.float32(4.0)) & (np.abs(Z) >= NEAR_TH)

    pts = np.stack([X, Y, Z, np.ones_like(Z)], axis=1)
    g = pts @ pose.T.astype(np.float32)
    gx, gy, gz = g[:, 0], g[:, 1], g[:, 2]
    gy = -gy + np.float32(0.0)

    mask = mask & (gy > 0) & (gy < 1)

    cells = int(ceil(40.0 / 0.1)) + 1
    shift = floor(cells / 2.0)
    grid_n = cells - 1
    iz = np.round(gz / np.float32(0.1) + np.float32(shift)).astype(np.int32)
    ix = np.round(gx / np.float32(0.1) + np.float32(shift)).astype(np.int32)
    inb = (iz >= 0) & (iz < grid_n) & (ix >= 0) & (ix < grid_n)
    wgt = (mask & inb).astype(np.float64)
    izc = np.clip(iz, 0, grid_n - 1)
    ixc = np.clip(ix, 0, grid_n - 1)
    flat = izc.astype(np.int64) * grid_n + ixc
    grid = np.bincount(flat, weights=wgt, minlength=grid_n * grid_n)
    return grid.reshape(grid_n, grid_n).astype(np.float32)


def kernel(depth, pose):
    depth = np.ascontiguousarray(np.asarray(depth), dtype=np.float32)
    pose = np.asarray(pose, dtype=np.float32)

    if not np.array_equal(pose, np.eye(4, dtype=np.float32)):
        return _numpy_reference(depth, pose)

    from concourse.bass_utils import run_bass_kernel_spmd

    nc = _get_nc()
    tj = _tj_table()
    in_maps = []
    for c in range(N_CORES):
        r0 = c * RPC
        in_maps.append(
            {
                "depth_slice": np.ascontiguousarray(depth[r0 : r0 + RPC, :]),
                "tj": np.ascontiguousarray(tj[r0 : r0 + RPC].reshape(RPC, 1)),
            }
        )

    res = run_bass_kernel_spmd(nc, in_maps, core_ids=list(range(N_CORES)))

    wins = _x_windows()
    wmax_wg = [
        max(wins[wg * (FW // FG) + k][1] for k in range(FW // FG))
        for wg in range(NWG)
    ]
    grid = np.zeros((GRID_N, GRID_N), np.float64)
    for r in res.results:
        arr = r["partial"]
        for g in range(NG):
            lo, wgn = wins[g]
            S = _stack_of(wmax_wg[g // (FW // FG)])
            a = arr[g, : S * wgn, : S * IZ_N].astype(np.float64)
            a = a.reshape(wgn, S, IZ_N, S)
            h = np.einsum("wfrf->wr", a)
            grid[IZ_LO : IZ_LO + IZ_N, lo : lo + wgn] += h.T
    return grid.astype(np.float32)


# revision 8
# speedup vs baseline: 1.0679x; 1.0679x over previous
"""Trainium2 Bass kernel (v4) for nn_DirectDepthMapper (histogram binning).

Identity-pose fast path, 8 cores x 256 rows (rows >= 2048 are always masked):
  * Quantize chains on ACT with fused round-via-(200+2^23) bias; masks as
    DVE scalar_tensor_tensor; Y = ix - lo_g rebase per 128-column group.
  * One-hot builds write CONTIGUOUS [128, 512] rows (strided short-run
    writes are 15-25x slower on real HW): W rows 0..27 on DVE is_equal
    (4x bf16 mode), rows 28..39 on ACT via Derivative_Erf(3*(sz-r)) which
    equals 1.125 * exact one-hot (+ ~1.4e-4 neighbor bleed) in bf16 — the
    host divides those rows back.  X one-hot rows on DVE.
  * One rearrange-based tensor_copy repacks the X one-hot to stack-major
    xs[p, st, w, f] so the matmul stationary is a single uniform-stride AP
    (walrus requires 1 free dim for weights; the moving side may have 2).
  * Stacked matmuls: S=8 (S=4 for the widest edge chunks) pixel-columns
    per matmul -> ~1280 matmuls/core instead of 8192 (real HW charges
    ~96ns/matmul instruction).  psum [S*wgn, S*40] accumulates per
    column-group; the f==f2 diagonal blocks are the real histogram,
    off-diagonal cells collect garbage in disjoint cells.  8-bank rotation.
  * Host: extract diagonals, scale ACT rows by 1/1.125, scatter windows.

Non-identity pose falls back to an exact numpy replica of the reference.
"""

import sys

sys.path.insert(0, "/opt/trn_rl_repo")

import numpy as np

H = 4096
W = 4096
GRID_N = 400
NEAR_TH = np.float32(0.1)

IZ_LO, IZ_N = 201, 40
IX_LO, IX_N = 180, 41

ROWS_USED = 2048
N_CORES = 8
RPC = ROWS_USED // N_CORES
PB = 128
NBLK = RPC // PB
FG = 128
NG = W // FG
FW = 512
NWG = W // FW
REPEAT = 1

W_DVE_ROWS = 40
SKIP_PE = False
MM_STRIDE = 1
U_ON_DVE = True
# W one-hot rows [IZ_N - W_ACT_ROWS, IZ_N) built on the ACT engine via
# Derivative_Erf(3*(sz - r)) = 1.125 * exact-one-hot + ~1.4e-4 neighbor
# bleed (device-probed).  Host divides those rows by ERF_PEAK.
W_ACT_ROWS = 12
ERF_PEAK = 1.125


def _stack_of(wmx):
    return 8 if wmx <= 16 else 4

TWO23 = np.float32(8388608.0)
B23 = float(200.0 + 8388608.0)
XSCALE = np.float32(10.0 / 4096.0)


_PATCHED = False


def _apply_tile_patch():
    global _PATCHED
    if _PATCHED:
        return
    import concourse.tile as tile_mod
    from concourse.vector_clock import ScopedClock, VectorClock

    n_procs = 27

    def _drain_and_barrier(self, tick_clock, wait_clock):
        g = tick_clock.global_clock
        procs = [p for p in range(n_procs) if g[p] > 0]
        for p in procs:
            vc = VectorClock([g[q] if q == p else 0 for q in range(n_procs)])
            d = self.nc.sync.drain()
            wait_clock.add_sem_waits(d.ins, ScopedClock({None: vc}))
        self.nc.all_engine_barrier()
        assert self.sems is not None
        popped = self.nc._tile_sem_poison_stack.pop()
        assert popped is self._sem_poison
        self.nc.clear_and_free_semaphores(list(self.sems.allocated().values()))
        self.nc.all_engine_barrier()

    tile_mod.TileContext._drain_and_barrier = _drain_and_barrier
    _PATCHED = True


def _split_multiwait(nc):
    import concourse.mybir as mybir

    for f in nc.m.functions:
        for bb in f.blocks:
            out = []
            changed = False
            for inst in bb.instructions:
                si = inst.sync_info
                waits = list(si.on_wait) if si is not None and si.on_wait else []
                if len(waits) > 1:
                    changed = True
                    for wi, w in enumerate(waits[:-1]):
                        nop = mybir.InstNoOp(
                            name=f"{inst.name}-sw{wi}",
                            engine=inst.engine,
                            sync_info=mybir.SyncInfo(on_wait=[w], on_update=[]),
                            bass_nofuse=True,
                        )
                        out.append(nop)
                    si.on_wait = [waits[-1]]
                out.append(inst)
            if changed:
                bb.instructions[:] = out


def _tj_table():
    tj = np.empty(ROWS_USED, np.float32)
    for j in range(ROWS_USED):
        m = 2047 - j
        if m <= 0:
            tj[j] = np.float32(-1.0)
            continue
        if m <= 1024:
            tj[j] = np.float32(4.0)
            continue
        t = np.float32(np.float64(4096.0) / m)
        while np.float32(t * np.float32(m)) >= np.float32(4096.0):
            t = np.nextafter(t, np.float32(-np.inf), dtype=np.float32)
        up = np.nextafter(t, np.float32(np.inf), dtype=np.float32)
        while np.float32(up * np.float32(m)) < np.float32(4096.0):
            t = up
            up = np.nextafter(t, np.float32(np.inf), dtype=np.float32)
        tj[j] = min(np.float32(4.0), up)
    return tj


def _x_windows():
    wins = []
    for g in range(NG):
        ci_min = g * FG - 2047
        ci_max = g * FG + FG - 1 - 2047
        glo = 200.0 + min(0.0, 40.0 * ci_min / 4096.0, 40.0 * ci_max / 4096.0)
        ghi = 200.0 + max(0.0, 40.0 * ci_min / 4096.0, 40.0 * ci_max / 4096.0)
        lo = int(np.clip(np.floor(glo), IX_LO, IX_LO + IX_N - 1))
        hi = int(np.clip(np.ceil(ghi), IX_LO, IX_LO + IX_N - 1))
        wins.append((lo, hi - lo + 1))
    return wins


def _build_bass():
    import concourse.bass as bass
    import concourse.mybir as mybir
    import concourse.tile as tile
    from contextlib import ExitStack

    dt = mybir.dt
    op = mybir.AluOpType
    AF = mybir.ActivationFunctionType

    nc = bass.Bass("TRN2", target_bir_lowering=False, debug=False)

    depth_in = nc.dram_tensor(
        "depth_slice", [RPC, W], dt.float32, kind="ExternalInput"
    )
    tj_in = nc.dram_tensor("tj", [RPC, 1], dt.float32, kind="ExternalInput")
    part_out = nc.dram_tensor(
        "partial", [NG, PB, IZ_N * 8], dt.float32, kind="ExternalOutput"
    )

    wins = _x_windows()
    wmax_wg = [
        max(wins[wg * (FW // FG) + k][1] for k in range(FW // FG))
        for wg in range(NWG)
    ]

    with tile.TileContext(nc) as tc:
        with ExitStack() as ctx:
            cpool = ctx.enter_context(tc.tile_pool(name="consts", bufs=1))
            dpool = ctx.enter_context(tc.tile_pool(name="depth", bufs=2))
            epool = ctx.enter_context(tc.tile_pool(name="etmp", bufs=2))
            bpool = ctx.enter_context(tc.tile_pool(name="bins", bufs=2))
            wpool = ctx.enter_context(tc.tile_pool(name="wonehot", bufs=2))
            # xt feeds only the same-engine (DVE) repack: bufs=1 is stall-free
            xpool = ctx.enter_context(tc.tile_pool(name="xonehot", bufs=1))
            ppool = ctx.enter_context(
                tc.tile_pool(name="psum", bufs=1, space="PSUM")
            )
            gpool = ctx.enter_context(tc.tile_pool(name="stage", bufs=2))
            spool2 = ctx.enter_context(tc.tile_pool(name="xstack", bufs=2))

            cx = cpool.tile([PB, W], dt.float32)
            lo_full = cpool.tile([PB, W], dt.bfloat16)
            with tc.tile_pool(name="setup", bufs=1) as spool:
                cx_i16 = spool.tile([PB, W], dt.int16)
                nc.gpsimd.iota(
                    cx_i16[:, :], pattern=[[1, W]], base=-2047,
                    channel_multiplier=0,
                )
                nc.vector.tensor_copy(cx[:, :], cx_i16[:, :])
            for g in range(NG):
                nc.gpsimd.memset(
                    lo_full[:, g * FG : (g + 1) * FG], float(wins[g][0])
                )
            wbias = cpool.tile([PB, max(1, W_ACT_ROWS)], dt.float32)
            for i in range(W_ACT_ROWS):
                r = IZ_N - W_ACT_ROWS + i
                nc.vector.memset(
                    wbias[:, i : i + 1], float(-3.0 * (IZ_LO + r))
                )

            tjs = []
            for b in range(NBLK):
                tjt = dpool.tile([PB, 1], dt.float32, tag=f"tj{b}")
                nc.sync.dma_start(
                    tjt[:, :], tj_in.ap()[b * PB : (b + 1) * PB, :]
                )
                tjs.append(tjt)

            psums = []
            for k in range(8):
                ps = ppool.tile(
                    [PB, IZ_N * 8], dt.float32, space="PSUM", tag=f"ps{k}",
                    name=f"psbank{k}",
                )
                if SKIP_PE:
                    nc.vector.memset(ps[:, :], 0.0)
                psums.append(ps)

            for rep in range(REPEAT):
                for wg in range(NWG):
                    wsl = slice(wg * FW, (wg + 1) * FW)
                    wmx = wmax_wg[wg]
                    S = _stack_of(wmx)
                    nstk = FW // S
                    spg = FG // S
                    for b in range(NBLK):
                        d = dpool.tile([PB, FW], dt.float32, tag=f"d{b}")
                        nc.sync.dma_start(
                            d[:, :],
                            depth_in.ap()[b * PB : (b + 1) * PB, wsl],
                        )
                        dsl = slice(0, FW)
                        u = epool.tile([PB, FW], dt.float32, tag="u")
                        ueng = nc.vector if U_ON_DVE else nc.gpsimd
                        ueng.tensor_tensor(
                            out=u[:, :], in0=d[:, dsl], in1=cx[:, wsl],
                            op=op.mult,
                        )
                        nc.scalar.activation(
                            u[:, :], u[:, :], AF.Copy, bias=B23,
                            scale=float(XSCALE),
                        )
                        sx = bpool.tile([PB, FW], dt.bfloat16, tag="sx")
                        nc.scalar.activation(
                            sx[:, :], u[:, :], AF.Copy, bias=-float(TWO23),
                            scale=1.0,
                        )
                        yt = bpool.tile([PB, FW], dt.bfloat16, tag="yt")
                        nc.vector.tensor_tensor(
                            out=yt[:, :], in0=sx[:, :], in1=lo_full[:, wsl],
                            op=op.subtract,
                        )
                        zq = epool.tile([PB, FW], dt.float32, tag="zq")
                        nc.scalar.activation(
                            zq[:, :], d[:, dsl], AF.Copy, bias=B23, scale=10.0
                        )
                        nc.scalar.activation(
                            zq[:, :], zq[:, :], AF.Copy, bias=-float(TWO23),
                            scale=1.0,
                        )
                        nc.vector.scalar_tensor_tensor(
                            out=zq[:, :], in0=d[:, dsl], scalar=tjs[b][:, 0:1],
                            in1=zq[:, :], op0=op.is_lt, op1=op.mult,
                        )
                        sz = bpool.tile([PB, FW], dt.bfloat16, tag="sz")
                        nc.vector.scalar_tensor_tensor(
                            out=sz[:, :], in0=d[:, dsl], scalar=float(NEAR_TH),
                            in1=zq[:, :], op0=op.is_ge, op1=op.mult,
                        )

                        wt = wpool.tile([PB, IZ_N, FW], dt.bfloat16, tag="wt")
                        for r in range(IZ_N - W_ACT_ROWS):
                            nc.vector.tensor_scalar(
                                wt[:, r, :], sz[:, :], float(IZ_LO + r), None,
                                op.is_equal,
                            )
                        for i in range(W_ACT_ROWS):
                            r = IZ_N - W_ACT_ROWS + i
                            nc.scalar.activation(
                                wt[:, r, :], sz[:, :], AF.Derivative_Erf,
                                bias=wbias[:, i : i + 1], scale=3.0,
                            )
                        xt = xpool.tile([PB, 21, FW], dt.bfloat16, tag="xt")
                        for w in range(wmx):
                            nc.vector.tensor_scalar(
                                xt[:, w, :], yt[:, :], float(w), None,
                                op.is_equal,
                            )

                        if SKIP_PE:
                            continue
                        # repack X one-hot to stack-major: xs[p, st, w, f]
                        # = xt[p, w, st*S+f]; stationary slice xs[:, st]
                        # is then one uniform-stride free dim.
                        xs = spool2.tile(
                            [PB, nstk, wmx, S], dt.bfloat16, tag="xs"
                        )
                        nc.vector.tensor_copy(
                            xs[:, :, :, :],
                            xt[:, 0:wmx, :].rearrange(
                                "p w (st f) -> p st w f", f=S
                            ),
                        )
                        # stacked matmuls: lhsT = xs[:, st] (S pixel-cols,
                        # w-major), rhs = wt[:, :, f0:f0+S] (2 free dims),
                        # psum rows w*S+f, cols r*S+f2; diag f==f2 real.
                        for k in range(FW // FG):
                            g = wg * (FW // FG) + k
                            lo, wgn = wins[g]
                            ps = psums[g % 8]
                            sis = [s for s in range(spg)
                                   if s % MM_STRIDE == 0]
                            for si in sis:
                                st = k * spg + si
                                f0 = st * S
                                nc.tensor.matmul(
                                    out=ps[: S * wgn, : S * IZ_N],
                                    lhsT=xs[:, st, 0:wgn, :],
                                    rhs=wt[:, :, f0 : f0 + S],
                                    start=(
                                        rep == 0 and b == 0 and si == sis[0]
                                    ),
                                    stop=(
                                        rep == REPEAT - 1
                                        and b == NBLK - 1
                                        and si == sis[-1]
                                    ),
                                )
                            if rep == REPEAT - 1 and b == NBLK - 1:
                                stage = gpool.tile(
                                    [PB, IZ_N * 8], dt.float32, tag="st"
                                )
                                nc.scalar.activation(
                                    stage[: S * wgn, : S * IZ_N],
                                    ps[: S * wgn, : S * IZ_N],
                                    AF.Copy, bias=0.0, scale=1.0,
                                )
                                nc.sync.dma_start(
                                    part_out.ap()[g, : S * wgn, : S * IZ_N],
                                    stage[: S * wgn, : S * IZ_N],
                                )

    _split_multiwait(nc)
    return nc


_NC_CACHE = None


def _get_nc():
    global _NC_CACHE
    if _NC_CACHE is None:
        _apply_tile_patch()
        _NC_CACHE = _build_bass()
    return _NC_CACHE


def _numpy_reference(depth, pose):
    from math import ceil, floor

    h, w = depth.shape
    fx, fy = np.float32(w), np.float32(h)
    cx, cy = w // 2 - 1, h // 2 - 1

    d = depth.T.reshape(-1)
    xv = np.repeat(np.arange(w, dtype=np.float32), h)
    yv = np.tile(np.arange(h, dtype=np.float32), w)
    X = d * (xv - np.float32(cx)) / fx
    Y = d * (yv - np.float32(cy)) / fy
    Z = d

    mask = (np.abs(Z) < np.float32(4.0)) & (np.abs(Z) >= NEAR_TH)

    pts = np.stack([X, Y, Z, np.ones_like(Z)], axis=1)
    g = pts @ pose.T.astype(np.float32)
    gx, gy, gz = g[:, 0], g[:, 1], g[:, 2]
    gy = -gy + np.float32(0.0)

    mask = mask & (gy > 0) & (gy < 1)

    cells = int(ceil(40.0 / 0.1)) + 1
    shift = floor(cells / 2.0)
    grid_n = cells - 1
    iz = np.round(gz / np.float32(0.1) + np.float32(shift)).astype(np.int32)
    ix = np.round(gx / np.float32(0.1) + np.float32(shift)).astype(np.int32)
    inb = (iz >= 0) & (iz < grid_n) & (ix >= 0) & (ix < grid_n)
    wgt = (mask & inb).astype(np.float64)
    izc = np.clip(iz, 0, grid_n - 1)
    ixc = np.clip(ix, 0, grid_n - 1)
    flat = izc.astype(np.int64) * grid_n + ixc
    grid = np.bincount(flat, weights=wgt, minlength=grid_n * grid_n)
    return grid.reshape(grid_n, grid_n).astype(np.float32)


def kernel(depth, pose):
    depth = np.ascontiguousarray(np.asarray(depth), dtype=np.float32)
    pose = np.asarray(pose, dtype=np.float32)

    if not np.array_equal(pose, np.eye(4, dtype=np.float32)):
        return _numpy_reference(depth, pose)

    from concourse.bass_utils import run_bass_kernel_spmd

    nc = _get_nc()
    tj = _tj_table()
    in_maps = []
    for c in range(N_CORES):
        r0 = c * RPC
        in_maps.append(
            {
                "depth_slice": np.ascontiguousarray(depth[r0 : r0 + RPC, :]),
                "tj": np.ascontiguousarray(tj[r0 : r0 + RPC].reshape(RPC, 1)),
            }
        )

    res = run_bass_kernel_spmd(nc, in_maps, core_ids=list(range(N_CORES)))

    wins = _x_windows()
    wmax_wg = [
        max(wins[wg * (FW // FG) + k][1] for k in range(FW // FG))
        for wg in range(NWG)
    ]
    # ACT-built W rows carry the Derivative_Erf peak factor
    rscale = np.ones(IZ_N, np.float64)
    rscale[IZ_N - W_ACT_ROWS :] = 1.0 / ERF_PEAK
    grid = np.zeros((GRID_N, GRID_N), np.float64)
    for r in res.results:
        arr = r["partial"]
        for g in range(NG):
            lo, wgn = wins[g]
            S = _stack_of(wmax_wg[g // (FW // FG)])
            a = arr[g, : S * wgn, : S * IZ_N].astype(np.float64)
            a = a.reshape(wgn, S, IZ_N, S)
            h = np.einsum("wfrf->wr", a) * rscale[None, :]
            grid[IZ_LO : IZ_LO + IZ_N, lo : lo + wgn] += h.T
    return grid.astype(np.float32)
